# revision 74
# baseline (speedup 1.0000x reference)
"""Trainium2 Bass kernel for relative-position multi-head attention.

Math (per batch element b, head h):
    k = key @ Wk.T + bk, q = query @ Wq.T + bq, v = value @ Wv.T + bv
    R = pe @ Wr.T + br                       # [2L, HID]; rpe[i,j] = R[j-i+L]
    A + C = (q + u_bias) @ k.T               # u folded into q
    B + D = skew((q + v_bias) @ R_h.T)       # skew: [i, dd] -> [i, j]
    score = (A+B+C+D)/sqrt(DH), mask keys j >= seq_len, softmax over j
    out = (attn @ v) @ Wf.T + bf

Device design (v3):
  - bf16 everywhere on the main path (inputs, weights, intermediates);
    PSUM accumulation stays fp32. R is host-precomputed (it depends only
    on module parameters), scaled by 64 and stored fp8 e4m3 (values are
    ~1e-2, below e4m3's normal range; the ident-add uses I/64 to undo).
  - "scores transposed" layout [j (partitions), i (free)]: the key mask is
    a per-partition bias on the exp (masks precomputed on host), the
    denominator comes from a ones column packed into v, and attn @ v
    needs no on-chip transposes.
  - The skew is a DRAM round-trip with a 512-wide window: row-tile t of
    S2 = (q+v)@R_h.T only needs rel positions [256-128t, 768-128t), so we
    write [128, 512] tiles at row stride 640 and read the skewed [j, i]
    view back with an XBAR-transposed stride-639 read. Heads are packed
    in pairs at stride L*639 so each pair needs one write + one transpose.
  - One NeuronCore per batch element (data-parallel over batch).
  - Deep pipeline: all projections and all skew round-trips are issued
    up front; the eight softmax/attn chains then stream while late skews
    are still in flight.
"""

import sys

try:
    import concourse.bass as bass  # noqa: F401
except ImportError:
    sys.path.insert(0, "/opt/trn_rl_repo")

import ml_dtypes
import numpy as np

import concourse.bass as bass
import concourse.tile as tile
from concourse import bacc, mybir
from concourse.bass_utils import run_bass_kernel_spmd

F32 = mybir.dt.float32
BF16 = mybir.dt.bfloat16
FP8 = mybir.dt.float8e4
AF = mybir.ActivationFunctionType
OP = mybir.AluOpType

B, L, HID, NH, DH = 8, 384, 512, 8, 64
DD = 2 * L          # 768 distinct relative positions
NT = L // 128       # 3 token tiles
CT = HID // 128     # 4 channel tiles
NP = NH // 2        # 4 head pairs
SCALE = 1.0 / 8.0   # 1/sqrt(DH)
NEG = -30000.0      # mask bias; exp(x*SCALE + NEG) == 0.0 in fp32

RSCALE = 64.0       # R stored as R*64 in fp8; ident = I/64 compensates
WIN = 512           # skew window width (needed cols per 128-row tile: 511)
WSTR = 640          # skew scratch row stride (in 2-byte units)
RSTR = WSTR - 1     # transposed read row stride
TSTR = 128 * WSTR - 128   # scratch offset between row tiles
SKSZ = L * WSTR     # scratch 2-byte units per head pair (heads byte-packed)

# w1 blob column offsets (bf16, partition-folded [128, .])
WQ_OFF = 0
W1A_COLS = CT * HID                   # 2048: [WqT] loads first
ONES_OFF = W1A_COLS
WK_OFF = ONES_OFF + 64
W1_COLS = WK_OFF + CT * HID           # 4160
IDENT_OFF = CT * DD                   # ident/64 lives in the fp8 blob
WV_OFF = 0
WF_OFF = CT * HID
W2_COLS = 2 * CT * HID                # 4096

WARMUP_MM = 8       # keep the PE p-state ramp alive through the load phase


def _build_program(skip_bias_rows: bool):
    nc = bacc.Bacc("TRN2", target_bir_lowering=False, debug=False, num_devices=8)

    def din(name, shape, dt):
        return nc.dram_tensor(name, shape, dt, kind="ExternalInput").ap()

    # per-core inputs (channel-major activations, prepped on host)
    qk = din("qk", [128, 2 * CT * L], BF16)      # [qT folded | kT folded]
    vt = din("vt", [128, CT * L], BF16)          # vT folded
    sm = din("sm", [128, 3 * CT + NT], F32)      # bias cols + mask cols
    # shared (replicated) weights, pre-transposed + partition-folded on host
    w1 = din("w1", [128, W1_COLS], BF16)         # [ident/64 | ones | WqT | WkT]
    w2 = din("w2", [128, W2_COLS], BF16)         # [WvT | WfT]
    r8 = din("r8", [128, CT * DD + 128], FP8)    # [R.T * 64, folded | I/64]
    rows = din("rows", [1, 128 + 2 * HID], BF16) if not skip_bias_rows else None

    out = nc.dram_tensor("out", [L, HID], BF16, kind="ExternalOutput").ap()
    skews = [nc.dram_tensor(f"skew{p}", [SKSZ], BF16) for p in range(NP)]

    io = dict(
        qk=qk, vt=vt, sm=sm, w1=w1, w2=w2, r8=r8, rows=rows, out=out,
        skews=skews, skip_bias_rows=skip_bias_rows,
    )
    with tile.TileContext(nc) as tc, nc.allow_low_precision(
        reason="bf16/fp8 mixed precision is intentional; PSUM accumulates fp32"
    ):
        _body(tc, io)
    nc.compile()
    return nc


def _body(tc, io):
    nc = tc.nc
    skip_bias_rows = io["skip_bias_rows"]

    from contextlib import ExitStack

    with ExitStack() as ctx:
        consts = ctx.enter_context(tc.tile_pool(name="consts", bufs=1))
        work = ctx.enter_context(tc.tile_pool(name="work", bufs=1))
        hpool = ctx.enter_context(tc.tile_pool(name="hpool", bufs=6))
        bt_pool = ctx.enter_context(tc.tile_pool(name="bt", bufs=4))
        exp_pool = ctx.enter_context(tc.tile_pool(name="exps", bufs=6))
        # PSUM budget (8 banks): psA 2 (proj + odd-pair s2 + final), psS 2
        # (even-pair s2), psT 2 (scores), psPV 2 (attn). Even/odd pairs get
        # independent matmul->copy rings so neither copy chain stalls the
        # other.
        psA = ctx.enter_context(tc.tile_pool(name="psA", bufs=2, space="PSUM"))
        psS = ctx.enter_context(tc.tile_pool(name="psS", bufs=2, space="PSUM"))
        psT = ctx.enter_context(tc.tile_pool(name="psT", bufs=2, space="PSUM"))
        psPV = ctx.enter_context(tc.tile_pool(name="psPV", bufs=2, space="PSUM"))

        # ---- PE warmup: garbage matmuls keep the p-state ramp alive while
        # the real operands stream in (results are never read) ----
        if WARMUP_MM:
            wtile = work.tile([128, 512], BF16, tag="wtile", name="wtile")
            nc.gpsimd.memset(wtile, 0.0)
            for i in range(WARMUP_MM):
                pw = psS.tile([128, WIN], F32, tag="s", name="pw")
                nc.tensor.matmul(pw, wtile[:, 0:128], wtile, start=True, stop=True)

        # ---- loads: split into first-use-ordered chunks ----
        csm = consts.tile([128, 3 * CT + NT], F32, tag="csm", name="csm")
        nc.sync.dma_start(out=csm, in_=io["sm"])
        cw1 = consts.tile([128, W1_COLS], BF16, tag="cw1", name="cw1")
        nc.sync.dma_start(out=cw1[:, 0:W1A_COLS], in_=io["w1"][:, 0:W1A_COLS])
        cqk = consts.tile([128, 2 * CT * L], BF16, tag="cqk", name="cqk")
        nc.sync.dma_start(out=cqk[:, 0 : CT * L], in_=io["qk"][:, 0 : CT * L])
        cr = consts.tile([128, CT * DD + 128], FP8, tag="cr", name="cr")
        nc.sync.dma_start(out=cr, in_=io["r8"])
        nc.sync.dma_start(out=cw1[:, W1A_COLS:], in_=io["w1"][:, W1A_COLS:])
        nc.sync.dma_start(out=cqk[:, CT * L :], in_=io["qk"][:, CT * L :])
        cv = consts.tile([128, CT * L], BF16, tag="cv", name="cv")
        nc.sync.dma_start(out=cv, in_=io["vt"])
        cw2 = consts.tile([128, W2_COLS], BF16, tag="cw2", name="cw2")
        nc.sync.dma_start(out=cw2[:, 0:WF_OFF], in_=io["w2"][:, 0:WF_OFF])
        rows_c = None
        if not skip_bias_rows:
            rows_c = consts.tile([1, 128 + 2 * HID], BF16, tag="rows", name="rows")
            nc.sync.dma_start(out=rows_c, in_=io["rows"])

        ident_c = cr[:, IDENT_OFF : IDENT_OFF + 128]
        ones64 = cw1[0:1, ONES_OFF : ONES_OFF + 64]

        def wq(kt):
            return cw1[:, WQ_OFF + kt * HID : WQ_OFF + (kt + 1) * HID]

        def wk(kt):
            return cw1[:, WK_OFF + kt * HID : WK_OFF + (kt + 1) * HID]

        def wv(kt):
            return cw2[:, WV_OFF + kt * HID : WV_OFF + (kt + 1) * HID]

        def wf(kt):
            return cw2[:, WF_OFF + kt * HID : WF_OFF + (kt + 1) * HID]

        def qts(kt):
            return cqk[:, kt * L : (kt + 1) * L]

        def kts(kt):
            return cqk[:, CT * L + kt * L : CT * L + (kt + 1) * L]

        def vts(kt):
            return cv[:, kt * L : (kt + 1) * L]

        def rview(mt, hs, c0, c1):
            return cr[hs, mt * DD + c0 : mt * DD + c1]

        def bias(mt, c):  # c: 0=bq+u, 1=bq+v, 2=bk
            return csm[:, mt * 3 + c : mt * 3 + c + 1]

        def mask(jt):
            return csm[:, 3 * CT + jt : 3 * CT + jt + 1]

        qu_cm, qv_cm, k_cm = [None] * CT, [None] * CT, [None] * CT
        sm_state = [None] * NH
        bt_tiles = [None] * NP
        s2b_tiles = [None] * NP
        v_ext = []
        ot_cm = [
            work.tile([128, L], BF16, tag=f"ot_cm{mt}", name=f"ot_cm{mt}")
            for mt in range(CT)
        ]


        def proj_q(mt):
            ms = slice(mt * 128, (mt + 1) * 128)
            ps = psA.tile([128, 512], F32, tag="psA", name="psq")
            for kt in range(CT):
                nc.tensor.matmul(
                    ps[:, 0:L], wq(kt)[:, ms], qts(kt),
                    start=(kt == 0), stop=(kt == CT - 1),
                )
            t = work.tile([128, L], BF16, tag=f"qu_cm{mt}", name=f"qu_cm{mt}")
            nc.vector.tensor_scalar(
                out=t, in0=ps[:, 0:L], scalar1=bias(mt, 0), scalar2=None, op0=OP.add
            )
            qu_cm[mt] = t
            t = work.tile([128, L], FP8, tag=f"qv_cm{mt}", name=f"qv_cm{mt}")
            nc.scalar.activation(
                out=t, in_=ps[:, 0:L], func=AF.Identity, bias=bias(mt, 1)
            )
            qv_cm[mt] = t

        def proj_k(mt):
            ms = slice(mt * 128, (mt + 1) * 128)
            ps = psA.tile([128, 512], F32, tag="psA", name="psk")
            for kt in range(CT):
                nc.tensor.matmul(
                    ps[:, 0:L], wk(kt)[:, ms], kts(kt),
                    start=(kt == 0), stop=(kt == CT - 1),
                )
            t = work.tile([128, L], BF16, tag=f"k_cm{mt}", name=f"k_cm{mt}")
            nc.vector.tensor_scalar(
                out=t, in0=ps[:, 0:L], scalar1=bias(mt, 2), scalar2=None, op0=OP.add
            )
            k_cm[mt] = t

        def proj_v():
            # v token-major, packed per head: [64 v cols][1 ones][1 pad] x 8.
            # The ones column folds the softmax denominator into attn @ v.
            for it in range(NT):
                isl = slice(it * 128, (it + 1) * 128)
                ps = psA.tile([128, 512], F32, tag="psA", name="psv")
                for kt in range(CT):
                    nc.tensor.matmul(
                        ps, vts(kt)[:, isl], wv(kt),
                        start=(kt == 0), stop=(kt == CT - 1) and skip_bias_rows,
                    )
                if not skip_bias_rows:
                    nc.tensor.matmul(
                        ps, rows_c[0:1, 0:128], rows_c[0:1, 128:640],
                        start=False, stop=True,
                    )
                t = work.tile([128, NH, 66], BF16, tag=f"v_ext{it}", name=f"v_ext{it}")
                nc.vector.tensor_copy(
                    out=t[:, :, 0:64], in_=ps.rearrange("p (h d) -> p h d", h=NH)
                )
                nc.gpsimd.memset(t[:, :, 64:65], 1.0)
                v_ext.append(t)


        def s2_pair(p):
            """S2 = (q + v_bias) @ (64*R_h).T windowed -> DRAM at row stride
            640 (heads of the pair at stride L*639): one write DMA per pair."""
            mt = p
            # even pairs: psS ring + DVE copies; odd pairs: psA ring + Act
            # copies -- two independent matmul->copy pipelines
            pool, tag = (psS, "s") if p % 2 == 0 else (psA, "psA")
            copy = nc.scalar.copy if p % 2 == 0 else nc.vector.tensor_copy
            # the two heads are byte-packed: each 2-byte scratch unit holds
            # (head0, head1) fp8 values for one (i, rel) -- so one write and
            # one XBAR transpose move BOTH heads at half the bf16 cost
            s2b = hpool.tile([128, NT, WIN], BF16, tag="s2b", name="s2b")
            s2b_tiles[p] = s2b
            s2b_f8 = s2b.bitcast(FP8).rearrange("p t (w two) -> p t w two", two=2)
            for hh in range(2):
                half = hh * 64
                hs = slice(half, half + 64)
                for it in range(NT):
                    isl = slice(it * 128, (it + 1) * 128)
                    a0 = 256 - 128 * it
                    ps2 = pool.tile([128, WIN], F32, tag=tag, name="ps2")
                    nc.tensor.matmul(
                        ps2, qv_cm[mt][hs, isl], rview(mt, hs, a0, a0 + WIN),
                        start=True, stop=True, tile_position=(half, 0),
                    )
                    copy(out=s2b_f8[:, it, :, hh], in_=ps2)
            sk = io["skews"][p]
            # spread the 8 skew DMAs over three queues (SP/Act/SWDGE) to
            # hide the per-queue inter-DMA completion-semaphore latency
            wr_eng = [nc.sync, nc.scalar, nc.sync, nc.scalar][p]
            wr_eng.dma_start(
                out=bass.AP(
                    tensor=sk, offset=0,
                    ap=[[WSTR, 128], [TSTR, NT], [1, WIN]],
                ),
                in_=bass.AP(
                    tensor=s2b.tensor, offset=s2b.offset,
                    ap=[list(s2b.ap[0]), [WIN, NT], [1, WIN]],
                ),
            )

        def s2_read(p):
            """One XBAR-transposed stride-639 read = 64*(B+D).T tiles
            [j (partitions), i] for both heads of the pair. The last pair is
            split per j-tile (queues are idle by then) so its softmax can
            start on jt=0 before the rest lands."""
            bt3 = bt_pool.tile([128, NT, L], BF16, tag="bt3", name="bt3")
            tr_eng = nc.sync if p % 2 == 0 else nc.scalar
            if True:
                for jt in range(NT):
                    (nc.sync if jt == 1 else nc.scalar).dma_start(
                        out=bt3[:, jt, :],
                        in_=bass.AP(
                            tensor=io["skews"][p],
                            offset=128 + jt * 128,
                            ap=[[RSTR, L], [1, 128]],
                        ),
                        transpose=True,
                    )
            else:
                tr_eng.dma_start(
                    out=bt3,
                    in_=bass.AP(
                        tensor=io["skews"][p], offset=128, ap=[[RSTR, L], [1, L]]
                    ),
                    transpose=True,
                )
            bt_tiles[p] = bt3

        def softmax_pv(h):
            """Scores (A+C via matmul, 64*(B+D) via I/64-accumulate), masked
            exp, attn @ v with folded denominator, then normalize."""
            mt, hh = h // 2, h % 2
            half = hh * 64
            hs = slice(half, half + 64)
            btp_f8 = bt_tiles[mt].bitcast(FP8).rearrange(
                "p t (l two) -> p t l two", two=2
            )
            exps = []
            for jt in range(NT):
                jsl = slice(jt * 128, (jt + 1) * 128)
                pst = psT.tile([128, L], F32, tag="st", name="pst")
                nc.tensor.matmul(
                    pst, k_cm[mt][hs, jsl], qu_cm[mt][hs, :],
                    start=True, stop=False, tile_position=(half, 0),
                )
                nc.tensor.matmul(
                    pst, ident_c, btp_f8[:, jt, :, hh],
                    start=False, stop=True,
                )
                e = exp_pool.tile([128, L], BF16, tag=f"exps{jt}", name=f"e{jt}")
                nc.scalar.activation(
                    out=e, in_=pst, func=AF.Exp, bias=mask(jt), scale=SCALE
                )
                exps.append(e)

            # attn @ v; psum rows 0..63 = out_h.T, row 64 = sum_j exp.
            # ppv alternates between two pools (psS is idle by now): 4
            # effective slots, so head h+2 never waits on head h's normalize
            ppv = (
                psPV.tile([65, L], F32, tag="pv", name="ppv")
                if h % 2 == 0
                else psS.tile([65, L], F32, tag="s", name="ppv")
            )
            for kt in range(NT):
                nc.tensor.matmul(
                    ppv, v_ext[kt][:, h, 0:65].opt(), exps[kt],
                    start=(kt == 0), stop=(kt == NT - 1),
                )
            rrow = hpool.tile([1, L], BF16, tag="rrow", name="rrow")
            nc.vector.reciprocal(out=rrow, in_=ppv[64:65, :])
            sm_state[h] = (ppv, rrow)

        def softmax_norm(h):
            """Deferred normalize: emitted one head later so the pbc matmul
            never blocks the in-order PE queue on this head's reciprocal."""
            mt, hh = h // 2, h % 2
            half = hh * 64
            hs = slice(half, half + 64)
            ppv, rrow = sm_state[h]
            # broadcast the denominator to 64 partitions on the idle Pool
            # engine, then a single DVE divide normalizes: the per-head DVE
            # cost is one op, so the normalize chains of successive heads
            # pipeline across Act -> Pool -> DVE
            rbc = hpool.tile([64, L], BF16, tag="rbc", name="rbc")
            nc.gpsimd.partition_broadcast(out_ap=rbc, in_ap=rrow)
            nc.vector.tensor_tensor(
                out=ot_cm[mt][hs, :], in0=ppv[0:64, :], in1=rbc, op=OP.mult
            )

        def final(it):
            isl = slice(it * 128, (it + 1) * 128)
            ps = psA.tile([128, 512], F32, tag="psA", name="psf")
            for kt in range(CT):
                nc.tensor.matmul(
                    ps, ot_cm[kt][:, isl], wf(kt),
                    start=(kt == 0), stop=(kt == CT - 1) and skip_bias_rows,
                )
            if not skip_bias_rows:
                nc.tensor.matmul(
                    ps, rows_c[0:1, 0:128], rows_c[0:1, 640:1152],
                    start=False, stop=True,
                )
            osb = hpool.tile([128, 512], BF16, tag="osb", name="osb")
            (nc.scalar.copy if it % 2 else nc.vector.tensor_copy)(out=osb, in_=ps)
            (nc.scalar if it != 1 else nc.sync).dma_start(
                out=io["out"][isl, :], in_=osb
            )

        # ---- deep pipeline: q-projections and skew round-trips issue up
        # front (k/v projections fill PE slack); softmax chains then stream
        # behind the transposed reads ----
        proj_q(0)
        proj_q(1)
        s2_pair(0)
        s2_pair(1)
        s2_read(0)
        proj_q(2)
        proj_q(3)
        s2_pair(2)
        s2_read(1)
        s2_pair(3)
        proj_k(0)
        proj_k(1)
        s2_read(2)
        proj_k(2)
        proj_k(3)
        s2_read(3)
        proj_v()
        # wf load is gated behind the last transpose via a WAW token so it
        # fills the DMA tail instead of displacing the skew stream
        nc.gpsimd.tensor_copy(
            out=cw2[0:1, WF_OFF : WF_OFF + 1], in_=bt_tiles[NP - 1][0:1, 0, 0:1]
        )
        nc.gpsimd.dma_start(out=cw2[:, WF_OFF:], in_=io["w2"][:, WF_OFF:])
        softmax_pv(0)
        softmax_pv(1)
        for h in range(2, NH):
            softmax_pv(h)
            softmax_norm(h - 2)
        softmax_norm(NH - 2)
        softmax_norm(NH - 1)
        for it in range(NT):
            final(it)


_CACHE = {}


def _get_nc(skip_bias_rows: bool):
    key = skip_bias_rows
    if key not in _CACHE:
        _CACHE[key] = _build_program(skip_bias_rows)
    return _CACHE[key]


def _fold(a):
    """[512, N] -> [128, 4*N]: row a*128+p, col n -> [p, a*N + n]."""
    n = a.shape[1]
    return np.ascontiguousarray(
        a.reshape(CT, 128, n).transpose(1, 0, 2).reshape(128, CT * n)
    )


def prep_in_maps(inputs):
    """Host-side sharding + layout marshaling. Returns (in_maps, skip_bias_rows)."""
    f = np.float32
    bf = ml_dtypes.bfloat16
    f8 = ml_dtypes.float8_e4m3
    g = {k: np.asarray(v) for k, v in inputs.items()}

    # R depends only on module parameters: precompute, scale into fp8 range.
    R = (g["pe"].astype(f) @ g["Wr"].astype(f).T) + g["br"].astype(f)  # [DD, HID]
    ident = np.zeros((128, 128), f)
    np.fill_diagonal(ident, 1.0 / RSCALE)
    r8 = np.concatenate(
        [_fold((R.T * RSCALE).astype(f8)), ident.astype(f8)], axis=1
    )

    ones = np.ones((128, 64), f)
    w1 = np.concatenate(
        [_fold(g["Wq"].T.astype(f)), ones, _fold(g["Wk"].T.astype(f))], axis=1
    ).astype(bf)
    w2 = np.concatenate(
        [_fold(g["Wv"].T.astype(f)), _fold(g["Wf"].T.astype(f))], axis=1
    ).astype(bf)

    bcols = np.stack(
        [
            g["bq"].astype(f) + g["u_bias"].astype(f).reshape(-1),
            g["bq"].astype(f) + g["v_bias"].astype(f).reshape(-1),
            g["bk"].astype(f),
        ],
        axis=1,
    )  # [512, 3]
    bfold = _fold(bcols)  # [128, 12]

    shared = {"w1": w1, "w2": w2, "r8": r8}
    skip_bias_rows = not (np.any(g["bv"]) or np.any(g["bf"]))
    if not skip_bias_rows:
        shared["rows"] = (
            np.concatenate([np.ones(128, f), g["bv"].astype(f), g["bf"].astype(f)])
            .reshape(1, -1)
            .astype(bf)
        )

    seq = np.asarray(g["seq_len"]).astype(np.int64)
    iota = np.arange(L)
    in_maps = []
    for b in range(B):
        m = dict(shared)
        m["qk"] = np.concatenate(
            [_fold(g["query"][b].T.astype(f)), _fold(g["key"][b].T.astype(f))], axis=1
        ).astype(bf)
        m["vt"] = _fold(g["value"][b].T.astype(f)).astype(bf)
        masks = ((iota >= seq[b]) * NEG).astype(f).reshape(NT, 128).T  # [128, 3]
        m["sm"] = np.concatenate([bfold, masks], axis=1).astype(f)
        in_maps.append(m)
    return in_maps, skip_bias_rows


def kernel(**inputs) -> np.ndarray:
    in_maps, skip_bias_rows = prep_in_maps(inputs)
    nc = _get_nc(skip_bias_rows)
    res = run_bass_kernel_spmd(nc, in_maps, list(range(B)))
    return np.stack([res.results[c]["out"] for c in range(B)]).astype(np.float32)


# revision 75
# speedup vs baseline: 1.2072x; 1.2072x over previous
"""Trainium2 Bass kernel for relative-position multi-head attention.

Math (per batch element b, head h):
    k = key @ Wk.T + bk, q = query @ Wq.T + bq, v = value @ Wv.T + bv
    R = pe @ Wr.T + br                       # [2L, HID]; rpe[i,j] = R[j-i+L]
    A + C = (q + u_bias) @ k.T               # u folded into q
    B + D = skew((q + v_bias) @ R_h.T)       # skew: [i, dd] -> [i, j]
    score = (A+B+C+D)/sqrt(DH), mask keys j >= seq_len, softmax over j
    out = (attn @ v) @ Wf.T + bf

Device design (v3):
  - bf16 everywhere on the main path (inputs, weights, intermediates);
    PSUM accumulation stays fp32. R is host-precomputed (it depends only
    on module parameters), scaled by 64 and stored fp8 e4m3 (values are
    ~1e-2, below e4m3's normal range; the ident-add uses I/64 to undo).
  - "scores transposed" layout [j (partitions), i (free)]: the key mask is
    a per-partition bias on the exp (masks precomputed on host), the
    denominator comes from a ones column packed into v, and attn @ v
    needs no on-chip transposes.
  - The skew is a DRAM round-trip with a 512-wide window: row-tile t of
    S2 = (q+v)@R_h.T only needs rel positions [256-128t, 768-128t), so we
    write [128, 512] tiles at row stride 640 and read the skewed [j, i]
    view back with an XBAR-transposed stride-639 read. Heads are packed
    in pairs at stride L*639 so each pair needs one write + one transpose.
  - One NeuronCore per batch element (data-parallel over batch).
  - Deep pipeline: all projections and all skew round-trips are issued
    up front; the eight softmax/attn chains then stream while late skews
    are still in flight.
"""

import sys

try:
    import concourse.bass as bass  # noqa: F401
except ImportError:
    sys.path.insert(0, "/opt/trn_rl_repo")

import ml_dtypes
import numpy as np

import concourse.bass as bass
import concourse.tile as tile
from concourse import bacc, mybir
from concourse.bass_utils import run_bass_kernel_spmd

F32 = mybir.dt.float32
BF16 = mybir.dt.bfloat16
FP8 = mybir.dt.float8e4
AF = mybir.ActivationFunctionType
OP = mybir.AluOpType

B, L, HID, NH, DH = 8, 384, 512, 8, 64
DD = 2 * L          # 768 distinct relative positions
NT = L // 128       # 3 token tiles
CT = HID // 128     # 4 channel tiles
NP = NH // 2        # 4 head pairs
SCALE = 1.0 / 8.0   # 1/sqrt(DH)
NEG = -30000.0      # mask bias; exp(x*SCALE + NEG) == 0.0 in fp32

RSCALE = 64.0       # R stored as R*64 in fp8; ident = I/64 compensates
WIN = 512           # skew window width (needed cols per 128-row tile: 511)
WSTR = 640          # skew scratch row stride (in 2-byte units)
RSTR = WSTR - 1     # transposed read row stride
TSTR = 128 * WSTR - 128   # scratch offset between row tiles
SKSZ = L * WSTR     # scratch 2-byte units per head pair (heads byte-packed)

# w1 blob column offsets (bf16, partition-folded [128, .])
WQ_OFF = 0
W1A_COLS = CT * HID                   # 2048: [WqT] loads first
ONES_OFF = W1A_COLS
WK_OFF = ONES_OFF + 64
W1_COLS = WK_OFF + CT * HID           # 4160
IDENT_OFF = CT * DD                   # ident/64 lives in the fp8 blob
WV_OFF = 0
WF_OFF = CT * HID
W2_COLS = 2 * CT * HID                # 4096

WARMUP_MM = 8       # keep the PE p-state ramp alive through the load phase


def _build_program(skip_bias_rows: bool):
    nc = bacc.Bacc("TRN2", target_bir_lowering=False, debug=False, num_devices=8)

    def din(name, shape, dt):
        return nc.dram_tensor(name, shape, dt, kind="ExternalInput").ap()

    # per-core inputs (channel-major activations, prepped on host)
    qk = din("qk", [128, 2 * CT * L], BF16)      # [qT folded | kT folded]
    vt = din("vt", [128, CT * L], BF16)          # vT folded
    sm = din("sm", [128, 3 * CT + NT], F32)      # bias cols + mask cols
    # shared (replicated) weights, pre-transposed + partition-folded on host
    w1 = din("w1", [128, W1_COLS], BF16)         # [ident/64 | ones | WqT | WkT]
    w2 = din("w2", [128, W2_COLS], BF16)         # [WvT | WfT]
    r8 = din("r8", [128, CT * DD + 128], FP8)    # [R.T * 64, folded | I/64]
    rows = din("rows", [1, 128 + 2 * HID], BF16) if not skip_bias_rows else None

    out = nc.dram_tensor("out", [L, HID], BF16, kind="ExternalOutput").ap()
    skews = [nc.dram_tensor(f"skew{p}", [SKSZ], BF16) for p in range(NP)]

    io = dict(
        qk=qk, vt=vt, sm=sm, w1=w1, w2=w2, r8=r8, rows=rows, out=out,
        skews=skews, skip_bias_rows=skip_bias_rows,
    )
    with tile.TileContext(nc) as tc, nc.allow_low_precision(
        reason="bf16/fp8 mixed precision is intentional; PSUM accumulates fp32"
    ):
        _body(tc, io)
    nc.compile()
    return nc


def _body(tc, io):
    nc = tc.nc
    skip_bias_rows = io["skip_bias_rows"]

    from contextlib import ExitStack

    with ExitStack() as ctx:
        consts = ctx.enter_context(tc.tile_pool(name="consts", bufs=1))
        work = ctx.enter_context(tc.tile_pool(name="work", bufs=1))
        hpool = ctx.enter_context(tc.tile_pool(name="hpool", bufs=6))
        bt_pool = ctx.enter_context(tc.tile_pool(name="bt", bufs=4))
        exp_pool = ctx.enter_context(tc.tile_pool(name="exps", bufs=6))
        # PSUM budget (8 banks): psA 2 (proj + odd-pair s2 + final), psS 2
        # (even-pair s2), psT 2 (scores), psPV 2 (attn). Even/odd pairs get
        # independent matmul->copy rings so neither copy chain stalls the
        # other.
        psA = ctx.enter_context(tc.tile_pool(name="psA", bufs=2, space="PSUM"))
        psS = ctx.enter_context(tc.tile_pool(name="psS", bufs=2, space="PSUM"))
        psT = ctx.enter_context(tc.tile_pool(name="psT", bufs=2, space="PSUM"))
        psPV = ctx.enter_context(tc.tile_pool(name="psPV", bufs=2, space="PSUM"))

        # ---- PE warmup: garbage matmuls keep the p-state ramp alive while
        # the real operands stream in (results are never read) ----
        if WARMUP_MM:
            wtile = work.tile([128, 512], BF16, tag="wtile", name="wtile")
            nc.gpsimd.memset(wtile, 0.0)
            for i in range(WARMUP_MM):
                pw = psS.tile([128, WIN], F32, tag="s", name="pw")
                nc.tensor.matmul(pw, wtile[:, 0:128], wtile, start=True, stop=True)

        # ---- loads: split into first-use-ordered chunks ----
        csm = consts.tile([128, 3 * CT + NT], F32, tag="csm", name="csm")
        nc.sync.dma_start(out=csm, in_=io["sm"])
        cw1 = consts.tile([128, W1_COLS], BF16, tag="cw1", name="cw1")
        nc.sync.dma_start(out=cw1[:, 0:W1A_COLS], in_=io["w1"][:, 0:W1A_COLS])
        cqk = consts.tile([128, 2 * CT * L], BF16, tag="cqk", name="cqk")
        nc.sync.dma_start(out=cqk[:, 0 : CT * L], in_=io["qk"][:, 0 : CT * L])
        cr = consts.tile([128, CT * DD + 128], FP8, tag="cr", name="cr")
        nc.sync.dma_start(out=cr, in_=io["r8"])
        nc.sync.dma_start(out=cw1[:, W1A_COLS:], in_=io["w1"][:, W1A_COLS:])
        nc.sync.dma_start(out=cqk[:, CT * L :], in_=io["qk"][:, CT * L :])
        cv = consts.tile([128, CT * L], BF16, tag="cv", name="cv")
        nc.sync.dma_start(out=cv, in_=io["vt"])
        cw2 = consts.tile([128, W2_COLS], BF16, tag="cw2", name="cw2")
        nc.sync.dma_start(out=cw2[:, 0:WF_OFF], in_=io["w2"][:, 0:WF_OFF])
        rows_c = None
        if not skip_bias_rows:
            rows_c = consts.tile([1, 128 + 2 * HID], BF16, tag="rows", name="rows")
            nc.sync.dma_start(out=rows_c, in_=io["rows"])

        ident_c = cr[:, IDENT_OFF : IDENT_OFF + 128]
        ones64 = cw1[0:1, ONES_OFF : ONES_OFF + 64]

        def wq(kt):
            return cw1[:, WQ_OFF + kt * HID : WQ_OFF + (kt + 1) * HID]

        def wk(kt):
            return cw1[:, WK_OFF + kt * HID : WK_OFF + (kt + 1) * HID]

        def wv(kt):
            return cw2[:, WV_OFF + kt * HID : WV_OFF + (kt + 1) * HID]

        def wf(kt):
            return cw2[:, WF_OFF + kt * HID : WF_OFF + (kt + 1) * HID]

        def qts(kt):
            return cqk[:, kt * L : (kt + 1) * L]

        def kts(kt):
            return cqk[:, CT * L + kt * L : CT * L + (kt + 1) * L]

        def vts(kt):
            return cv[:, kt * L : (kt + 1) * L]

        def rview(mt, hs, c0, c1):
            return cr[hs, mt * DD + c0 : mt * DD + c1]

        def bias(mt, c):  # c: 0=bq+u, 1=bq+v, 2=bk
            return csm[:, mt * 3 + c : mt * 3 + c + 1]

        def mask(jt):
            return csm[:, 3 * CT + jt : 3 * CT + jt + 1]

        qu_cm, qv_cm, k_cm = [None] * CT, [None] * CT, [None] * CT
        sm_state = [None] * NH
        bt_tiles = [None] * NP
        s2b_tiles = [None] * NP
        v_ext = []
        ot_cm = [
            work.tile([128, L], BF16, tag=f"ot_cm{mt}", name=f"ot_cm{mt}")
            for mt in range(CT)
        ]


        def proj_q(mt):
            ms = slice(mt * 128, (mt + 1) * 128)
            ps = psA.tile([128, 512], F32, tag="psA", name="psq")
            for kt in range(CT):
                nc.tensor.matmul(
                    ps[:, 0:L], wq(kt)[:, ms], qts(kt),
                    start=(kt == 0), stop=(kt == CT - 1),
                )
            t = work.tile([128, L], BF16, tag=f"qu_cm{mt}", name=f"qu_cm{mt}")
            nc.vector.tensor_scalar(
                out=t, in0=ps[:, 0:L], scalar1=bias(mt, 0), scalar2=None, op0=OP.add
            )
            qu_cm[mt] = t
            t = work.tile([128, L], FP8, tag=f"qv_cm{mt}", name=f"qv_cm{mt}")
            nc.scalar.activation(
                out=t, in_=ps[:, 0:L], func=AF.Identity, bias=bias(mt, 1)
            )
            qv_cm[mt] = t

        def proj_k(mt):
            ms = slice(mt * 128, (mt + 1) * 128)
            ps = psA.tile([128, 512], F32, tag="psA", name="psk")
            for kt in range(CT):
                nc.tensor.matmul(
                    ps[:, 0:L], wk(kt)[:, ms], kts(kt),
                    start=(kt == 0), stop=(kt == CT - 1),
                )
            t = work.tile([128, L], BF16, tag=f"k_cm{mt}", name=f"k_cm{mt}")
            nc.vector.tensor_scalar(
                out=t, in0=ps[:, 0:L], scalar1=bias(mt, 2), scalar2=None, op0=OP.add
            )
            k_cm[mt] = t

        def proj_v():
            # v token-major, packed per head: [64 v cols][1 ones][1 pad] x 8.
            # The ones column folds the softmax denominator into attn @ v.
            for it in range(NT):
                isl = slice(it * 128, (it + 1) * 128)
                ps = psA.tile([128, 512], F32, tag="psA", name="psv")
                for kt in range(CT):
                    nc.tensor.matmul(
                        ps, vts(kt)[:, isl], wv(kt),
                        start=(kt == 0), stop=(kt == CT - 1) and skip_bias_rows,
                    )
                if not skip_bias_rows:
                    nc.tensor.matmul(
                        ps, rows_c[0:1, 0:128], rows_c[0:1, 128:640],
                        start=False, stop=True,
                    )
                t = work.tile([128, NH, 66], BF16, tag=f"v_ext{it}", name=f"v_ext{it}")
                nc.vector.tensor_copy(
                    out=t[:, :, 0:64], in_=ps.rearrange("p (h d) -> p h d", h=NH)
                )
                nc.gpsimd.memset(t[:, :, 64:65], 1.0)
                v_ext.append(t)


        def s2_pair(p):
            """S2 = (q + v_bias) @ (64*R_h).T windowed -> DRAM at row stride
            640 (heads of the pair at stride L*639): one write DMA per pair."""
            mt = p
            # even pairs: psS ring + DVE copies; odd pairs: psA ring + Act
            # copies -- two independent matmul->copy pipelines
            pool, tag = (psS, "s") if p % 2 == 0 else (psA, "psA")
            copy = nc.scalar.copy if p % 2 == 0 else nc.vector.tensor_copy
            # the two heads are byte-packed: each 2-byte scratch unit holds
            # (head0, head1) fp8 values for one (i, rel) -- so one write and
            # one XBAR transpose move BOTH heads at half the bf16 cost
            s2b = hpool.tile([128, NT, WIN], BF16, tag="s2b", name="s2b")
            s2b_tiles[p] = s2b
            s2b_f8 = s2b.bitcast(FP8).rearrange("p t (w two) -> p t w two", two=2)
            for hh in range(2):
                half = hh * 64
                hs = slice(half, half + 64)
                for it in range(NT):
                    isl = slice(it * 128, (it + 1) * 128)
                    a0 = 256 - 128 * it
                    ps2 = pool.tile([128, WIN], F32, tag=tag, name="ps2")
                    nc.tensor.matmul(
                        ps2, qv_cm[mt][hs, isl], rview(mt, hs, a0, a0 + WIN),
                        start=True, stop=True, tile_position=(half, 0),
                    )
                    copy(out=s2b_f8[:, it, :, hh], in_=ps2)
            sk = io["skews"][p]
            # spread the 8 skew DMAs over three queues (SP/Act/SWDGE) to
            # hide the per-queue inter-DMA completion-semaphore latency
            wr_eng = [nc.sync, nc.scalar, nc.sync, nc.scalar][p]
            wr_eng.dma_start(
                out=bass.AP(
                    tensor=sk, offset=0,
                    ap=[[WSTR, 128], [TSTR, NT], [1, WIN]],
                ),
                in_=bass.AP(
                    tensor=s2b.tensor, offset=s2b.offset,
                    ap=[list(s2b.ap[0]), [WIN, NT], [1, WIN]],
                ),
            )

        def s2_read(p):
            """One XBAR-transposed stride-639 read = 64*(B+D).T tiles
            [j (partitions), i] for both heads of the pair. The last pair is
            split per j-tile (queues are idle by then) so its softmax can
            start on jt=0 before the rest lands."""
            bt3 = bt_pool.tile([128, NT, L], BF16, tag="bt3", name="bt3")
            tr_eng = nc.sync if p % 2 == 0 else nc.scalar
            if p == NP - 1:
                for jt in range(NT):
                    (nc.sync if jt == 1 else nc.scalar).dma_start(
                        out=bt3[:, jt, :],
                        in_=bass.AP(
                            tensor=io["skews"][p],
                            offset=128 + jt * 128,
                            ap=[[RSTR, L], [1, 128]],
                        ),
                        transpose=True,
                    )
            else:
                tr_eng.dma_start(
                    out=bt3,
                    in_=bass.AP(
                        tensor=io["skews"][p], offset=128, ap=[[RSTR, L], [1, L]]
                    ),
                    transpose=True,
                )
            bt_tiles[p] = bt3

        def softmax_pv(h):
            """Scores (A+C via matmul, 64*(B+D) via I/64-accumulate), masked
            exp, attn @ v with folded denominator, then normalize."""
            mt, hh = h // 2, h % 2
            half = hh * 64
            hs = slice(half, half + 64)
            btp_f8 = bt_tiles[mt].bitcast(FP8).rearrange(
                "p t (l two) -> p t l two", two=2
            )
            exps = []
            for jt in range(NT):
                jsl = slice(jt * 128, (jt + 1) * 128)
                pst = psT.tile([128, L], F32, tag="st", name="pst")
                nc.tensor.matmul(
                    pst, k_cm[mt][hs, jsl], qu_cm[mt][hs, :],
                    start=True, stop=False, tile_position=(half, 0),
                )
                nc.tensor.matmul(
                    pst, ident_c, btp_f8[:, jt, :, hh],
                    start=False, stop=True,
                )
                e = exp_pool.tile([128, L], BF16, tag=f"exps{jt}", name=f"e{jt}")
                nc.scalar.activation(
                    out=e, in_=pst, func=AF.Exp, bias=mask(jt), scale=SCALE
                )
                exps.append(e)

            # attn @ v; psum rows 0..63 = out_h.T, row 64 = sum_j exp.
            # ppv alternates between two pools (psS is idle by now): 4
            # effective slots, so head h+2 never waits on head h's normalize
            ppv = (
                psPV.tile([65, L], F32, tag="pv", name="ppv")
                if h % 2 == 0
                else psS.tile([65, L], F32, tag="s", name="ppv")
            )
            for kt in range(NT):
                nc.tensor.matmul(
                    ppv, v_ext[kt][:, h, 0:65].opt(), exps[kt],
                    start=(kt == 0), stop=(kt == NT - 1),
                )
            rrow = hpool.tile([1, L], BF16, tag="rrow", name="rrow")
            nc.vector.reciprocal(out=rrow, in_=ppv[64:65, :])
            sm_state[h] = (ppv, rrow)

        def softmax_norm(h):
            """Deferred normalize: emitted one head later so the pbc matmul
            never blocks the in-order PE queue on this head's reciprocal."""
            mt, hh = h // 2, h % 2
            half = hh * 64
            hs = slice(half, half + 64)
            ppv, rrow = sm_state[h]
            # broadcast the denominator to 64 partitions on the idle Pool
            # engine, then a single DVE divide normalizes: the per-head DVE
            # cost is one op, so the normalize chains of successive heads
            # pipeline across Act -> Pool -> DVE
            rbc = hpool.tile([64, L], BF16, tag="rbc", name="rbc")
            nc.gpsimd.partition_broadcast(out_ap=rbc, in_ap=rrow)
            nc.vector.tensor_tensor(
                out=ot_cm[mt][hs, :], in0=ppv[0:64, :], in1=rbc, op=OP.mult
            )

        def final(it):
            isl = slice(it * 128, (it + 1) * 128)
            ps = psA.tile([128, 512], F32, tag="psA", name="psf")
            for kt in range(CT):
                nc.tensor.matmul(
                    ps, ot_cm[kt][:, isl], wf(kt),
                    start=(kt == 0), stop=(kt == CT - 1) and skip_bias_rows,
                )
            if not skip_bias_rows:
                nc.tensor.matmul(
                    ps, rows_c[0:1, 0:128], rows_c[0:1, 640:1152],
                    start=False, stop=True,
                )
            osb = hpool.tile([128, 512], BF16, tag="osb", name="osb")
            (nc.scalar.copy if it % 2 else nc.vector.tensor_copy)(out=osb, in_=ps)
            (nc.scalar if it != 1 else nc.sync).dma_start(
                out=io["out"][isl, :], in_=osb
            )

        # ---- deep pipeline: q-projections and skew round-trips issue up
        # front (k/v projections fill PE slack); softmax chains then stream
        # behind the transposed reads ----
        proj_q(0)
        proj_q(1)
        s2_pair(0)
        s2_pair(1)
        s2_read(0)
        proj_q(2)
        proj_q(3)
        s2_pair(2)
        s2_read(1)
        s2_pair(3)
        proj_k(0)
        proj_k(1)
        s2_read(2)
        proj_k(2)
        proj_k(3)
        s2_read(3)
        proj_v()
        # wf load is gated behind the last transpose via a WAW token so it
        # fills the DMA tail instead of displacing the skew stream
        nc.gpsimd.tensor_copy(
            out=cw2[0:1, WF_OFF : WF_OFF + 1], in_=bt_tiles[NP - 1][0:1, 0, 0:1]
        )
        nc.gpsimd.dma_start(out=cw2[:, WF_OFF:], in_=io["w2"][:, WF_OFF:])
        softmax_pv(0)
        softmax_pv(1)
        for h in range(2, NH):
            softmax_pv(h)
            softmax_norm(h - 2)
        softmax_norm(NH - 2)
        softmax_norm(NH - 1)
        for it in range(NT):
            final(it)


_CACHE = {}


def _get_nc(skip_bias_rows: bool):
    key = skip_bias_rows
    if key not in _CACHE:
        _CACHE[key] = _build_program(skip_bias_rows)
    return _CACHE[key]


def _fold(a):
    """[512, N] -> [128, 4*N]: row a*128+p, col n -> [p, a*N + n]."""
    n = a.shape[1]
    return np.ascontiguousarray(
        a.reshape(CT, 128, n).transpose(1, 0, 2).reshape(128, CT * n)
    )


def prep_in_maps(inputs):
    """Host-side sharding + layout marshaling. Returns (in_maps, skip_bias_rows)."""
    f = np.float32
    bf = ml_dtypes.bfloat16
    f8 = ml_dtypes.float8_e4m3
    g = {k: np.asarray(v) for k, v in inputs.items()}

    # R depends only on module parameters: precompute, scale into fp8 range.
    R = (g["pe"].astype(f) @ g["Wr"].astype(f).T) + g["br"].astype(f)  # [DD, HID]
    ident = np.zeros((128, 128), f)
    np.fill_diagonal(ident, 1.0 / RSCALE)
    r8 = np.concatenate(
        [_fold((R.T * RSCALE).astype(f8)), ident.astype(f8)], axis=1
    )

    ones = np.ones((128, 64), f)
    w1 = np.concatenate(
        [_fold(g["Wq"].T.astype(f)), ones, _fold(g["Wk"].T.astype(f))], axis=1
    ).astype(bf)
    w2 = np.concatenate(
        [_fold(g["Wv"].T.astype(f)), _fold(g["Wf"].T.astype(f))], axis=1
    ).astype(bf)

    bcols = np.stack(
        [
            g["bq"].astype(f) + g["u_bias"].astype(f).reshape(-1),
            g["bq"].astype(f) + g["v_bias"].astype(f).reshape(-1),
            g["bk"].astype(f),
        ],
        axis=1,
    )  # [512, 3]
    bfold = _fold(bcols)  # [128, 12]

    shared = {"w1": w1, "w2": w2, "r8": r8}
    skip_bias_rows = not (np.any(g["bv"]) or np.any(g["bf"]))
    if not skip_bias_rows:
        shared["rows"] = (
            np.concatenate([np.ones(128, f), g["bv"].astype(f), g["bf"].astype(f)])
            .reshape(1, -1)
            .astype(bf)
        )

    seq = np.asarray(g["seq_len"]).astype(np.int64)
    iota = np.arange(L)
    in_maps = []
    for b in range(B):
        m = dict(shared)
        m["qk"] = np.concatenate(
            [_fold(g["query"][b].T.astype(f)), _fold(g["key"][b].T.astype(f))], axis=1
        ).astype(bf)
        m["vt"] = _fold(g["value"][b].T.astype(f)).astype(bf)
        masks = ((iota >= seq[b]) * NEG).astype(f).reshape(NT, 128).T  # [128, 3]
        m["sm"] = np.concatenate([bfold, masks], axis=1).astype(f)
        in_maps.append(m)
    return in_maps, skip_bias_rows


def kernel(**inputs) -> np.ndarray:
    in_maps, skip_bias_rows = prep_in_maps(inputs)
    nc = _get_nc(skip_bias_rows)
    res = run_bass_kernel_spmd(nc, in_maps, list(range(B)))
    return np.stack([res.results[c]["out"] for c in range(B)]).astype(np.float32)
